# revision 88
# baseline (speedup 1.0000x reference)
"""Trainium2 Bass kernel for a single attention layer.

Problem: x[4,2048,512], W_q/W_k/W_v[512,512], b_q/b_k/b_v[512]
  q = x@W_q+b_q; k = x@W_k+b_k; v = x@W_v+b_v
  out = softmax(q @ k.T) @ v          (per batch)

Sharding: 8 cores = 4 batches x 2 sequence-halves (data parallel).
Each core receives its batch's full x with its query-half rolled to the
front (key order is permutation-invariant under softmax-attention), and
computes the output rows for its 1024 queries.

Per-core design (PE-bound; all big matmuls fp32r = 1 cyc/row @512 free):
  identity for PE transposes built on-chip (Pool memset+affine_select)
  xT[d,s]    via PE transpose of x tiles   (evictions alternate DVE/ACT)
  KT[e,s] =  W_k.T @ xT   (+b_k on ACT eviction, fp32r)
  QT[e,q] =  W_q.T @ xT   (+b_q on ACT eviction, fp32r)
  V[s,e]  =  xT.T @ W_v   (DVE eviction -> bf16, bias folded into output)
  phase A is a wavefront: qt0 score chunks + early V tiles fill the
  DMA-arrival and eviction-latency gaps between T(kc) and K(kc).
  per 128-query tile, 2-deep software pipeline:
    scores chunks -> PSUM (4-buf pool), per-chunk row max (DVE),
    negmax (DVE), exp on ACT (bias=-max, accum rowsum) -> P bf16,
    P^T via DMA xbar transpose (14ns/tile, off the PE; split in halves
    for the last tile so its attend starts on the first half),
    attn@V (PT bf16 stationary x V bf16 moving) -> PSUM,
    final eviction fuses *1/rowsum + b_v in one DVE
    scalar_tensor_tensor, stores stream on SP during phase B.

HW notes baked in: GPSIMD cannot touch PSUM; vector ops read at most one
PSUM operand; matmul operands must share 32-bitness (identity is f32r via
affine_select, not bitcast — f32r inputs must be produced rounded);
DMA-completion semaphores cost 900ns; every DMA burns ~650ns of issue
latency on its queue.
"""
import sys

sys.path.insert(0, "/opt/trn_rl_repo")

import numpy as np
from contextlib import ExitStack

B, S, D = 4, 2048, 512
SQ = S // 2          # queries per core
P = 128              # partitions
DT = D // P          # 4 d-tiles
NT = S // P          # 16 s-tiles
QT_N = SQ // P       # 8 q-tiles per core
KC = S // 512        # 4 key chunks of 512
N_CORES = 8

# tail scheduling knobs (swept in sim)
SPLIT_XBAR_QTS = (7,)
H0_IN_LOOP = True          # emit first half right after exp chunk 1
SPLIT_ATTEND_QTS = (6, 7)  # qts whose attend runs as two 8-mm groups

_NC_CACHE = None


def _build_nc(reps=1):
    import concourse.bacc as bacc
    import concourse.tile as tile
    from concourse import mybir
    import concourse.bass as bass

    f32 = mybir.dt.float32
    f32r = mybir.dt.float32r
    bf16 = mybir.dt.bfloat16
    AF = mybir.ActivationFunctionType
    X = mybir.AxisListType.X

    nc = bacc.Bacc(trn_type="TRN2")

    x_d = nc.dram_tensor("xk", [S, D], f32r, kind="ExternalInput")
    wq_d = nc.dram_tensor("wq", [D, D], f32r, kind="ExternalInput")
    wk_d = nc.dram_tensor("wk", [D, D], f32r, kind="ExternalInput")
    wv_d = nc.dram_tensor("wv", [D, D], f32r, kind="ExternalInput")
    bq_d = nc.dram_tensor("bq", [D], f32, kind="ExternalInput")
    bk_d = nc.dram_tensor("bk", [D], f32, kind="ExternalInput")
    bv_d = nc.dram_tensor("bv", [D], f32, kind="ExternalInput")
    out_d = nc.dram_tensor("out", [SQ, D], f32, kind="ExternalOutput")

    with tile.TileContext(nc) as tc, ExitStack() as ctx:
        persist = ctx.enter_context(tc.tile_pool(name="persist", bufs=1))
        xload = ctx.enter_context(tc.tile_pool(name="xload", bufs=8))
        ppool = ctx.enter_context(tc.tile_pool(name="ppool", bufs=3))
        ptpool = ctx.enter_context(tc.tile_pool(name="ptpool", bufs=5))
        opool = ctx.enter_context(tc.tile_pool(name="opool", bufs=8))
        stat = ctx.enter_context(tc.tile_pool(name="stat", bufs=3))
        psS = ctx.enter_context(tc.tile_pool(name="psS", bufs=4,
                                             space="PSUM"))
        psA = ctx.enter_context(tc.tile_pool(name="psA", bufs=3,
                                             space="PSUM"))
        psPO = ctx.enter_context(tc.tile_pool(name="psPO", bufs=1,
                                              space="PSUM"))

        for _rep in range(reps):
            # identity for PE transposes built on-chip (Pool) — keeps the
            # serial DMA pipe free for x/W; each DMA costs ~650ns of issue
            # latency regardless of size
            ident_f = persist.tile([P, P], f32)
            nc.gpsimd.memset(ident_f, 1.0)
            identr = persist.tile([P, P], f32r)
            nc.gpsimd.affine_select(
                out=identr, in_=ident_f, pattern=[[1, P]],
                compare_op=mybir.AluOpType.is_equal, fill=0.0,
                base=0, channel_multiplier=-1)

            # ---- input DMAs in consumption order (single in-order pipe):
            # x chunk 0 + wk first (gates K(0)), biases slotted right
            # before their first eviction use, bv broadcast last.
            bk_sb = persist.tile([P, DT], f32)
            bq_sb = persist.tile([P, DT], f32)

            xc_tiles = [None] * NT
            w_sb = {}
            w_ap = {"wk": wk_d.ap().rearrange("(t p) e -> p t e", p=P),
                    "wq": wq_d.ap().rearrange("(t p) e -> p t e", p=P),
                    "wv": wv_d.ap().rearrange("(t p) e -> p t e", p=P)}
            for name in ("wk", "wq", "wv"):
                w_sb[name] = persist.tile([P, DT, D], f32r, tag=f"w_{name}",
                                          name=f"w_{name}")

            def load_x(st):
                xc = xload.tile([P, D], f32r, tag="xc")
                nc.sync.dma_start(out=xc, in_=x_d.ap()[st * P:(st + 1) * P, :])
                xc_tiles[st] = xc

            def load_w_dt(name, dt):
                nc.sync.dma_start(out=w_sb[name][:, dt, :], in_=w_ap[name][:, dt, :])

            for i in range(4):
                load_x(i)
                load_w_dt("wk", i)
            nc.sync.dma_start(out=bk_sb, in_=bk_d.ap().rearrange("(t p) -> p t", p=P))
            for i in range(4):
                load_w_dt("wq", i)
            nc.sync.dma_start(out=bq_sb, in_=bq_d.ap().rearrange("(t p) -> p t", p=P))
            for i in range(8):
                load_x(4 + i)
            for i in range(4):
                load_w_dt("wv", i)
            for i in range(4):
                load_x(12 + i)
            bv_bcast = persist.tile([P, D], f32)
            bv_ap = bass.AP(tensor=bv_d, offset=0, ap=[[0, P], [1, D]])
            nc.sync.dma_start(out=bv_bcast, in_=bv_ap)

            # ---- persistent SBUF operands ------------------------------
            xT = persist.tile([P, DT, S], f32r)
            KT = persist.tile([P, DT, S], f32r)
            QT = persist.tile([P, DT, SQ], f32r)
            V = persist.tile([P, NT, D], bf16)

            def emit_transposes(kc):
                # evictions alternate DVE/ACT (GPSIMD cannot read PSUM on
                # hardware) so the last lands sooner after the transposes
                evictors = [nc.vector.tensor_copy, nc.scalar.copy,
                            nc.vector.tensor_copy, nc.scalar.copy]
                for sti in range(4):
                    st = kc * 4 + sti
                    pst = psA.tile([P, 4 * P], f32r, tag="ps")
                    for dt in range(DT):
                        nc.tensor.transpose(
                            pst[:, dt * P:(dt + 1) * P],
                            xc_tiles[st][:, dt * P:(dt + 1) * P], identr)
                    evictors[sti](
                        out=xT[:, 0:DT, st * P:(st + 1) * P],
                        in_=pst.rearrange("p (t q) -> p t q", t=DT))

            def emit_proj_chunk(wname, dst, bias, kc):
                # dst[e, kc*512:(kc+1)*512] = W.T @ xT chunk (+bias on evict)
                for et in range(DT):
                    pp = psA.tile([P, 512], f32, tag="ps")
                    for dt in range(DT):
                        nc.tensor.matmul(
                            pp,
                            w_sb[wname][:, dt, et * P:(et + 1) * P],
                            xT[:, dt, kc * 512:(kc + 1) * 512],
                            start=(dt == 0), stop=(dt == DT - 1),
                        )
                    nc.scalar.activation(
                        out=dst[:, et, kc * 512:(kc + 1) * 512], in_=pp,
                        func=AF.Identity, bias=bias[:, et:et + 1], scale=1.0,
                    )

            def emit_v_proj(st_lo, st_hi):
                for st in range(st_lo, st_hi):
                    pp = psA.tile([P, 512], f32, tag="ps")
                    for dt in range(DT):
                        nc.tensor.matmul(
                            pp,
                            xT[:, dt, st * P:(st + 1) * P],
                            w_sb["wv"][:, dt, :],
                            start=(dt == 0), stop=(dt == DT - 1),
                        )
                    nc.vector.tensor_copy(out=V[:, st, :], in_=pp)

            state = {}
            recip_of = {}
            mx_of = {}

            def emit_scores_chunk(qt, kcc):
                ss = psS.tile([P, 512], f32, tag="ps")
                for et in range(DT):
                    nc.tensor.matmul(
                        ss,
                        QT[:, et, qt * P:(qt + 1) * P],
                        KT[:, et, kcc * 512:(kcc + 1) * 512],
                        start=(et == 0), stop=(et == DT - 1),
                    )
                # per-chunk row max on DVE (hardware allows only one PSUM
                # input per vector instruction, so no fused pairwise max)
                if qt not in mx_of:
                    mx_of[qt] = stat.tile([P, KC], f32, tag="mx",
                                          name=f"mx{qt}")
                nc.vector.reduce_max(
                    out=mx_of[qt][:, kcc:kcc + 1], in_=ss, axis=X)
                state.setdefault(qt, []).append(ss)

            def emit_scores(qt):
                for kcc in range(KC):
                    emit_scores_chunk(qt, kcc)

            def emit_softmax(qt, split_xbar=False):
                sc = state.pop(qt)
                negmax = stat.tile([P, 1], f32, tag="negmax")
                nc.vector.reduce_max(out=negmax, in_=mx_of.pop(qt), axis=X,
                                     negate=True)
                rs_part = stat.tile([P, KC], f32, tag="rs")
                p_sb = ppool.tile([P, S], bf16, tag="P")
                PT = ptpool.tile([P, NT, P], bf16, tag="PT")
                for kcc in range(KC):
                    nc.scalar.activation(
                        out=p_sb[:, kcc * 512:(kcc + 1) * 512], in_=sc[kcc],
                        func=AF.Exp, bias=negmax, scale=1.0,
                        accum_out=rs_part[:, kcc:kcc + 1],
                    )
                    if split_xbar and H0_IN_LOOP and kcc == 1:
                        nc.sync.dma_start_transpose(
                            PT[:, 0:NT // 2, :], p_sb[:, 0:S // 2])
                if split_xbar:
                    if not H0_IN_LOOP:
                        nc.sync.dma_start_transpose(
                            PT[:, 0:NT // 2, :], p_sb[:, 0:S // 2])
                    nc.sync.dma_start_transpose(
                        PT[:, NT // 2:NT, :], p_sb[:, S // 2:S])
                else:
                    nc.sync.dma_start_transpose(PT, p_sb)
                rowsum = stat.tile([P, 1], f32, tag="rowsum")
                nc.vector.reduce_sum(out=rowsum, in_=rs_part, axis=X)
                recip = stat.tile([P, 1], f32, tag="recip")
                nc.vector.reciprocal(recip, rowsum)
                state[("pt", qt)] = PT
                recip_of[qt] = recip

            def emit_attend_mm(qt, lo, hi):
                PT = state[("pt", qt)]
                if ("po", qt) not in state:
                    # alternate PSUM rings so back-to-back attends never
                    # wait on the previous eviction
                    po_pool = psPO if qt % 2 == 0 else psA
                    state[("po", qt)] = po_pool.tile(
                        [P, D], f32, tag="po" if qt % 2 == 0 else "ps",
                        name=f"po{qt}")
                po = state[("po", qt)]
                for kt in range(lo, hi):
                    nc.tensor.matmul(
                        po, PT[:, kt, :], V[:, kt, :],
                        start=(kt == 0), stop=(kt == NT - 1),
                        skip_group_check=True,
                    )

            def emit_attend_fin(qt):
                state.pop(("pt", qt))
                po = state.pop(("po", qt))
                o_sb = opool.tile([P, D], f32, tag="o")
                nc.vector.scalar_tensor_tensor(
                    out=o_sb, in0=po, scalar=recip_of[qt], in1=bv_bcast,
                    op0=mybir.AluOpType.mult, op1=mybir.AluOpType.add,
                )
                # store inline on SP: it dispatches during phase B (after
                # this qt's stt), keeping the HWDGE queue credits fresh so
                # the tail transposes are not blocked behind store backlog
                nc.sync.dma_start(
                    out=out_d.ap()[qt * P:(qt + 1) * P, :], in_=o_sb)

            def emit_attend(qt, split=False):
                if split:
                    emit_attend_mm(qt, 0, NT // 2)
                    emit_attend_mm(qt, NT // 2, NT)
                else:
                    emit_attend_mm(qt, 0, NT)
                emit_attend_fin(qt)

            # ---- phase A wavefront: transposes + K/Q projections, with
            # qt0 score chunks and early V tiles covering eviction and
            # DMA-arrival latency between each T(kc) and K(kc) ------------
            out_tiles = []
            emit_transposes(0)
            emit_proj_chunk("wk", KT, bk_sb, 0)
            emit_proj_chunk("wq", QT, bq_sb, 0)
            emit_transposes(1)
            emit_scores_chunk(0, 0)
            emit_proj_chunk("wk", KT, bk_sb, 1)
            emit_proj_chunk("wq", QT, bq_sb, 1)
            emit_scores_chunk(0, 1)
            emit_transposes(2)
            emit_v_proj(0, 4)
            emit_proj_chunk("wk", KT, bk_sb, 2)
            emit_transposes(3)
            emit_scores_chunk(0, 2)
            emit_v_proj(4, 8)
            emit_proj_chunk("wk", KT, bk_sb, 3)
            emit_scores_chunk(0, 3)

            # ---- phase B: 2-deep pipelined attention --------------------
            # PE order: V s(1) s(2) a(0) s(3) a(1) ... s(7) a(5) a(6) a(7)
            emit_softmax(0)
            emit_v_proj(8, NT)
            emit_scores(1)
            emit_softmax(1)
            for qt in range(2, QT_N):
                emit_scores(qt)
                emit_softmax(qt, split_xbar=(qt in SPLIT_XBAR_QTS))
                emit_attend(qt - 2, split=((qt - 2) in SPLIT_ATTEND_QTS))
            # tail: the last attend runs in halves so its first 8 matmuls
            # start as soon as the first half-transpose lands (the xbar
            # completion sem alone costs 900ns)
            emit_attend(QT_N - 2, split=(QT_N - 2) in SPLIT_ATTEND_QTS)
            emit_attend(QT_N - 1, split=(QT_N - 1) in SPLIT_ATTEND_QTS)


    nc.finalize()
    return nc


def _shard_inputs(x, W_q, W_k, W_v, b_q, b_k, b_v):
    in_maps = []
    for c in range(N_CORES):
        b, h = divmod(c, 2)
        xb = x[b]
        xk = xb if h == 0 else np.concatenate([xb[SQ:], xb[:SQ]], axis=0)
        in_maps.append({
            "xk": np.ascontiguousarray(xk),
            "wq": W_q, "wk": W_k, "wv": W_v,
            "bq": b_q, "bk": b_k, "bv": b_v,
        })
    return in_maps


def kernel(x, W_q, W_k, W_v, b_q, b_k, b_v):
    from concourse.bass_utils import run_bass_kernel_spmd

    global _NC_CACHE
    if _NC_CACHE is None:
        _NC_CACHE = _build_nc()
    nc = _NC_CACHE

    args = [np.ascontiguousarray(np.asarray(a, dtype=np.float32))
            for a in (x, W_q, W_k, W_v, b_q, b_k, b_v)]
    in_maps = _shard_inputs(*args)

    res = run_bass_kernel_spmd(nc, in_maps, core_ids=list(range(N_CORES))).results

    out = np.empty((B, S, D), dtype=np.float32)
    for c in range(N_CORES):
        b, h = divmod(c, 2)
        out[b, h * SQ:(h + 1) * SQ] = res[c]["out"]
    return out
